# revision 12
# baseline (speedup 1.0000x reference)
"""Bass/Trainium2 kernel for nn_CombinedModel (ROI extract + pool + conv +
multi-head classifier + CE loss), data-parallel over ROIs on 8 NeuronCores.

Self-contained: hardcodes shapes; takes full unsharded inputs, returns the
full output tuple (out_prov, out_alpha, out_ad, mask, total).
"""
import sys
for _p in ('/opt/pypackages', '/opt/trn_rl_repo'):
    if _p not in sys.path:
        sys.path.insert(0, _p)

import os
import types

import numpy as np
import ml_dtypes

import concourse.bacc as bacc
import concourse.tile as tile
import concourse.mybir as mybir
import concourse.bass as bass
from concourse.bass import ds
from concourse.bass_utils import run_bass_kernel_spmd
from concourse.masks import make_identity

F32 = mybir.dt.float32
BF16 = mybir.dt.bfloat16
I32 = mybir.dt.int32
AF = mybir.ActivationFunctionType
ALU = mybir.AluOpType
AX = mybir.AxisListType

# Problem constants
N_CORES = 8
N_ROIS = 1024
R = N_ROIS // N_CORES          # 128 rois per core
ROI_H, ROI_W = 112, 224
BANDS = 7
M = R * BANDS                  # 896 band-rows per core
C_IN = 3
PH, PW = 8, 16                 # adaptive pool output (per band)
KIN = C_IN * PH * PW           # 384  (conv matmul contraction, + 1 bias row)
NCONV = 16 * PH * PW           # 2048 (conv output: 16 oc x 128 spatial)
FLAT = 512                     # 16 oc x 4 x 8 after 2x2 pool
HID = 128
NSEG = 8                       # prov, alpha, ad0..5
SEGW = 38                      # padded per-segment logit width
PROV, ALPHA, AD = 38, 25, 35
CONF = 0.25
NEG = -1.0e30

ROIS_PER_TILE = 18             # 18*7 = 126 partitions per crop tile
CROP_TILES = [ROIS_PER_TILE] * 7 + [R - 7 * ROIS_PER_TILE]   # 7x18 + 2
MT = M // 128                  # 7 m-tiles of 128

IMG_HW = 1024 * 1024
MAX_OFF = (1024 - ROI_H - 1) * 1024 + (1024 - ROI_W - 1)


def build_nc():
    nc = bacc.Bacc("TRN2", target_bir_lowering=False, debug=False)

    x = nc.dram_tensor("x", [1, C_IN, 1024, 1024], F32, kind="ExternalInput")
    rois = nc.dram_tensor("rois", [R, 4], I32, kind="ExternalInput")
    det_t = nc.dram_tensor("det_t", [M, 1], F32, kind="ExternalInput")
    cls_t = nc.dram_tensor("cls_t", [M, NSEG], F32, kind="ExternalInput")
    bigw = nc.dram_tensor("bigw", [C_IN, 128, NCONV], BF16, kind="ExternalInput")
    bigwb = nc.dram_tensor("bigwb", [1, NCONV], BF16, kind="ExternalInput")
    w1 = nc.dram_tensor("w1", [8, FLAT, HID], F32, kind="ExternalInput")
    b1 = nc.dram_tensor("b1", [8, HID], F32, kind="ExternalInput")
    w2p = nc.dram_tensor("w2p", [HID, NSEG, SEGW], F32, kind="ExternalInput")
    b2p = nc.dram_tensor("b2p", [NSEG, SEGW], F32, kind="ExternalInput")

    o_prov = nc.dram_tensor("o_prov", [M, PROV], F32, kind="ExternalOutput")
    o_alpha = nc.dram_tensor("o_alpha", [M, ALPHA], F32, kind="ExternalOutput")
    o_ad = nc.dram_tensor("o_ad", [6, M, AD], F32, kind="ExternalOutput")
    o_mask = nc.dram_tensor("o_mask", [M, 1], F32, kind="ExternalOutput")
    o_total = nc.dram_tensor("o_total", [1, 1], F32, kind="ExternalOutput")

    from contextlib import ExitStack
    with tile.TileContext(nc) as tc, ExitStack() as ctx:
        consts = ctx.enter_context(tc.tile_pool(name="consts", bufs=1))
        crop_p = ctx.enter_context(tc.tile_pool(name="crop", bufs=2))
        wred_p = ctx.enter_context(tc.tile_pool(name="wred", bufs=2))
        pool_p = ctx.enter_context(tc.tile_pool(name="pooled", bufs=2))
        big_p = ctx.enter_context(tc.tile_pool(name="big", bufs=1))
        work_p = ctx.enter_context(tc.tile_pool(name="work", bufs=3))
        ce_p = ctx.enter_context(tc.tile_pool(name="ce", bufs=3))
        ps_conv = ctx.enter_context(tc.tile_pool(name="psc", bufs=2, space="PSUM"))
        ps_tr = ctx.enter_context(tc.tile_pool(name="pst", bufs=2, space="PSUM"))
        ps_h = ctx.enter_context(tc.tile_pool(name="psh", bufs=2, space="PSUM"))
        ps_lg = ctx.enter_context(tc.tile_pool(name="psl", bufs=2, space="PSUM"))

        # ---------- constants / weights into SBUF ----------
        ident = consts.tile([128, 128], F32)
        make_identity(nc, ident)

        rois_sb = consts.tile([1, R * 4], I32)
        nc.gpsimd.dma_start(out=rois_sb[:], in_=rois[:].rearrange("r f -> (r f)")[None, :])
        offs = consts.tile([1, R], I32)
        rv = rois_sb[:].rearrange("one (r f) -> one r f", f=4)
        tmp_y = consts.tile([1, R], I32)
        nc.vector.tensor_scalar(
            out=tmp_y[:][:, :, None], in0=rv[:, :, 1:2],
            scalar1=10, scalar2=None, op0=ALU.arith_shift_left)
        nc.vector.tensor_tensor(
            out=offs[:][:, :, None],
            in0=tmp_y[:][:, :, None],
            in1=rv[:, :, 0:1], op=ALU.add)

        bigw_sb = consts.tile([128, C_IN, NCONV], BF16)
        nc.gpsimd.dma_start(out=bigw_sb[:], in_=bigw[:].rearrange("c d n -> d c n"))
        bigwb_sb = consts.tile([1, NCONV], BF16)
        nc.gpsimd.dma_start(out=bigwb_sb[:], in_=bigwb[:])
        w1_sb = consts.tile([128, 8, 4, HID], F32)
        nc.gpsimd.dma_start(out=w1_sb[:], in_=w1[:].rearrange("k (q d) h -> d k q h", d=128))
        b1_sb = consts.tile([1, 8, HID], F32)
        nc.gpsimd.dma_start(out=b1_sb[:], in_=b1[:].rearrange("k h -> (k h)")[None, :].rearrange("one (k h) -> one k h", k=8))
        w2_sb = consts.tile([128, NSEG, SEGW], F32)
        nc.gpsimd.dma_start(out=w2_sb[:], in_=w2p[:])
        b2_sb = consts.tile([128, NSEG, SEGW], F32)
        nc.gpsimd.dma_start(
            out=b2_sb[:],
            in_=bass.AP(tensor=b2p, offset=0,
                        ap=[[0, 128], [SEGW, NSEG], [1, SEGW]]))

        ones_bf = consts.tile([1, M], BF16)
        nc.vector.memset(ones_bf[:], 1.0)
        ones_f = consts.tile([1, M], F32)
        nc.vector.memset(ones_f[:], 1.0)
        ones_col = consts.tile([128, 1], F32)
        nc.vector.memset(ones_col[:], 1.0)

        iota_i = consts.tile([128, SEGW], I32)
        nc.gpsimd.iota(iota_i[:], pattern=[[1, SEGW]], base=0, channel_multiplier=0)
        iota_f = consts.tile([128, SEGW], F32)
        nc.vector.tensor_copy(iota_f[:], iota_i[:])

        acc = consts.tile([128, 1], F32)
        nc.vector.memset(acc[:], 0.0)

        pooledT = big_p.tile([128, C_IN, M], BF16)
        flatT = big_p.tile([128, 4, M], F32)

        # ---------- stage A/B/C: gather crops, pool, transpose ----------
        xf = x[:].rearrange("one c h w -> (one c) (h w)")
        m_base = 0
        for t, nroi in enumerate(CROP_TILES):
            pc = nroi * BANDS
            crop = crop_p.tile([126, C_IN, 16, ROI_W], F32, tag="crop")
            for j in range(nroi):
                r_idx = t * ROIS_PER_TILE + j
                reg = nc.sync.alloc_register(f"off_{r_idx}")
                nc.sync.reg_load(reg, offs[0:1, r_idx:r_idx + 1])
                off = nc.sync.snap(reg, donate=True, min_val=0, max_val=MAX_OFF)
                src = xf[:, ds(off, ROI_H * 1024)]
                src = src.rearrange("c (r w) -> c r w", w=1024)[:, :, 0:ROI_W]
                src = src.rearrange("c (b r) w -> c b r w", b=BANDS)
                for cch in range(C_IN):
                    nc.sync.dma_start(
                        out=crop[j * BANDS:(j + 1) * BANDS, cch],
                        in_=src[cch])

            # pool: 14-wide max then row-pair max -> [pc, 3, 8, 16]
            wred = wred_p.tile([126, C_IN, 16, PW], F32, tag="wred")
            nc.vector.reduce_max(
                out=wred[:pc], axis=AX.X,
                in_=crop[:pc].rearrange("p c r (w k) -> p c r w k", k=14))
            pooled = pool_p.tile([126, C_IN, PH, PW], F32, tag="pooled")
            wv = wred[:pc].rearrange("p c (i rp) w -> p c i rp w", rp=2)
            nc.vector.tensor_tensor(
                out=pooled[:pc], in0=wv[:, :, :, 0, :], in1=wv[:, :, :, 1, :],
                op=ALU.max)

            # transpose each channel chunk: [pc, 128] -> [128, pc]
            pv = pooled[:].rearrange("p c i w -> p (c i w)")
            for q in range(C_IN):
                trp = ps_tr.tile([128, 128], F32, tag="tr")
                nc.tensor.transpose(
                    out=trp[:, :pc], in_=pv[:pc, q * 128:(q + 1) * 128],
                    identity=ident[:pc, :pc])
                nc.scalar.copy(out=pooledT[:, q, m_base:m_base + pc],
                               in_=trp[:, :pc])
            m_base += pc

        # ---------- stage D: conv as K=385 matmul + 2x2 pool + transpose ----------
        for mt in range(MT):
            ms = mt * 128
            for n in range(4):   # N chunks of 512 over NCONV=2048
                cps = ps_conv.tile([128, 512], F32, tag="cps")
                for q in range(C_IN):
                    nc.tensor.matmul(
                        out=cps[:], lhsT=pooledT[:, q, ms:ms + 128],
                        rhs=bigw_sb[:, q, n * 512:(n + 1) * 512],
                        start=(q == 0), stop=False)
                nc.tensor.matmul(
                    out=cps[:], lhsT=ones_bf[0:1, ms:ms + 128],
                    rhs=bigwb_sb[0:1, n * 512:(n + 1) * 512],
                    start=False, stop=True)
                # 2x2 maxpool over (i,j): [128m, 4o,8i,16j] -> [128m, 4o,4i,8j]
                t1 = work_p.tile([128, 4, PH, PW // 2], F32, tag="t1")
                cv = cps[:].rearrange("p (o i w rp) -> p o i w rp", o=4, i=PH, w=PW // 2)
                nc.vector.tensor_copy(t1[:], cv[:, :, :, :, 0])
                nc.vector.tensor_tensor(
                    out=t1[:], in0=cv[:, :, :, :, 1], in1=t1[:],
                    op=ALU.max)
                t2 = work_p.tile([128, 4, PH // 2, PW // 2], F32, tag="t2")
                tv = t1[:].rearrange("p o (i rp) w -> p o i rp w", rp=2)
                nc.vector.tensor_tensor(
                    out=t2[:], in0=tv[:, :, :, 0, :], in1=tv[:, :, :, 1, :],
                    op=ALU.max)
                # transpose [128m, 128d] -> [128d, 128m], relu on copy-out
                trp2 = ps_tr.tile([128, 128], F32, tag="tr")
                nc.tensor.transpose(
                    out=trp2[:], in_=t2[:].rearrange("p o i w -> p (o i w)"),
                    identity=ident[:])
                nc.scalar.activation(out=flatT[:, n, ms:ms + 128], in_=trp2[:],
                                     func=AF.Relu)

        # ---------- stage E: h = relu(flat @ w1[k] + b1[k]) for 8 heads ----------
        for mt in range(MT):
            ms = mt * 128
            hT = work_p.tile([128, 8, 128], F32, tag="hT")
            for g in range(2):           # head groups of 4 per PSUM bank
                hps = ps_h.tile([128, 512], F32, tag="hps")
                for kk in range(4):
                    k = g * 4 + kk
                    for q in range(4):
                        nc.tensor.matmul(
                            out=hps[:, kk * 128:(kk + 1) * 128],
                            lhsT=w1_sb[:, k, q, :], rhs=flatT[:, q, ms:ms + 128],
                            start=(q == 0), stop=False)
                    nc.tensor.matmul(
                        out=hps[:, kk * 128:(kk + 1) * 128],
                        lhsT=b1_sb[0:1, k, :], rhs=ones_f[0:1, ms:ms + 128],
                        start=False, stop=True)
                nc.scalar.activation(
                    out=hT[:].rearrange("p k h -> p (k h)")[:, g * 512:(g + 1) * 512],
                    in_=hps[:], func=AF.Relu)

            # ---------- stage F: head logits (padded to [8, 38]) ----------
            lps = ps_lg.tile([128, NSEG, SEGW], F32, tag="lps")
            for s in range(NSEG):
                nc.tensor.matmul(
                    out=lps[:, s, :], lhsT=hT[:, s, :], rhs=w2_sb[:, s, :],
                    start=True, stop=True)
            logits = ce_p.tile([128, NSEG, SEGW], F32, tag="logits")
            nc.vector.tensor_tensor(out=logits[:], in0=lps[:], in1=b2_sb[:],
                                    op=ALU.add)

            # outputs
            nc.scalar.dma_start(out=o_prov[ms:ms + 128, :], in_=logits[:, 0, 0:PROV])
            nc.scalar.dma_start(out=o_alpha[ms:ms + 128, :], in_=logits[:, 1, 0:ALPHA])
            for k in range(6):
                nc.scalar.dma_start(
                    out=o_ad[k, ms:ms + 128, :],
                    in_=logits[:, 2 + k, 0:AD])

            # ---------- stage G: CE / mask / loss ----------
            dt_sb = ce_p.tile([128, 1], F32, tag="dt")
            nc.scalar.dma_start(out=dt_sb[:], in_=det_t[ms:ms + 128, :])
            ct_sb = ce_p.tile([128, NSEG], F32, tag="ct")
            nc.scalar.dma_start(out=ct_sb[:], in_=cls_t[ms:ms + 128, :])

            mx = ce_p.tile([128, NSEG], F32, tag="mx")
            nc.vector.reduce_max(out=mx[:], in_=logits[:], axis=AX.X)
            cent = ce_p.tile([128, NSEG, SEGW], F32, tag="cent")
            nc.vector.tensor_tensor(
                out=cent[:], in0=logits[:],
                in1=mx[:][:, :, None].to_broadcast([128, NSEG, SEGW]),
                op=ALU.subtract)
            exps = ce_p.tile([128, NSEG, SEGW], F32, tag="exps")
            nc.scalar.activation(out=exps[:], in_=cent[:], func=AF.Exp)
            se = ce_p.tile([128, NSEG], F32, tag="se")
            nc.vector.reduce_sum(out=se[:], in_=exps[:], axis=AX.X)
            lnse = ce_p.tile([128, NSEG], F32, tag="lnse")
            nc.scalar.activation(out=lnse[:], in_=se[:], func=AF.Ln)
            lse = ce_p.tile([128, NSEG], F32, tag="lse")
            nc.vector.tensor_tensor(out=lse[:], in0=lnse[:], in1=mx[:], op=ALU.add)

            iob = iota_f[:][:, None, :].to_broadcast([128, NSEG, SEGW])
            oh = ce_p.tile([128, NSEG, SEGW], F32, tag="oh")
            nc.vector.tensor_tensor(
                out=oh[:], in0=iob,
                in1=ct_sb[:][:, :, None].to_broadcast([128, NSEG, SEGW]),
                op=ALU.is_equal)
            ohl = ce_p.tile([128, NSEG, SEGW], F32, tag="ohl")
            nc.vector.tensor_tensor(out=ohl[:], in0=oh[:], in1=logits[:], op=ALU.mult)
            picked = ce_p.tile([128, NSEG], F32, tag="picked")
            nc.vector.reduce_sum(out=picked[:], in_=ohl[:], axis=AX.X)

            # det loss on prov segment
            ohd = ce_p.tile([128, SEGW], F32, tag="ohd")
            nc.vector.tensor_tensor(
                out=ohd[:], in0=iota_f[:],
                in1=dt_sb[:].to_broadcast([128, SEGW]), op=ALU.is_equal)
            ohdl = ce_p.tile([128, SEGW], F32, tag="ohdl")
            nc.vector.tensor_tensor(out=ohdl[:], in0=ohd[:], in1=logits[:, 0, :],
                                    op=ALU.mult)
            pd = ce_p.tile([128, 1], F32, tag="pd")
            nc.vector.reduce_sum(out=pd[:], in_=ohdl[:], axis=AX.X)
            det_loss = ce_p.tile([128, 1], F32, tag="det_loss")
            nc.vector.tensor_tensor(out=det_loss[:], in0=lse[:, 0:1], in1=pd[:],
                                    op=ALU.subtract)
            msk = ce_p.tile([128, 1], F32, tag="msk")
            nc.vector.tensor_scalar(out=msk[:], in0=det_loss[:], scalar1=CONF,
                                    scalar2=None, op0=ALU.is_lt)
            nc.scalar.dma_start(out=o_mask[ms:ms + 128, :], in_=msk[:])

            celt = ce_p.tile([128, NSEG], F32, tag="celt")
            nc.vector.tensor_tensor(out=celt[:], in0=lse[:], in1=picked[:],
                                    op=ALU.subtract)
            cls_loss = ce_p.tile([128, 1], F32, tag="cls_loss")
            nc.vector.reduce_sum(out=cls_loss[:], in_=celt[:], axis=AX.X)
            contrib = ce_p.tile([128, 1], F32, tag="contrib")
            nc.vector.tensor_tensor(out=contrib[:], in0=msk[:], in1=cls_loss[:],
                                    op=ALU.mult)
            nc.vector.tensor_tensor(out=acc[:], in0=acc[:], in1=contrib[:],
                                    op=ALU.add)

        # ---------- stage H: partition-sum of acc -> total ----------
        tps = ps_tr.tile([128, 1], F32, tag="tr")
        nc.tensor.matmul(out=tps[0:1, :], lhsT=acc[:], rhs=ones_col[:],
                         start=True, stop=True)
        tot_sb = ce_p.tile([1, 1], F32, tag="tot_sb")
        nc.vector.tensor_copy(tot_sb[:], tps[0:1, :])
        nc.sync.dma_start(out=o_total[:], in_=tot_sb[:])

    nc.compile()
    return nc


_NC = None
LAST_EXEC_NS = None
_TRACE_READY = False


def _install_trace_shim():
    """Enable NTFF profiling under axon (no antenv.axon_hooks in this image)."""
    global _TRACE_READY
    if _TRACE_READY:
        return
    try:
        hooks = types.ModuleType("antenv.axon_hooks")
        hooks._h = None
        hooks.set_axon_ntff_profile_hook = lambda h: setattr(hooks, "_h", h)
        hooks.get_axon_ntff_profile_hook = lambda: hooks._h
        sys.modules["antenv.axon_hooks"] = hooks
        import antenv
        antenv.axon_hooks = hooks
        from trn_agent_boot.trn_boot import _ntff_profile_via_ctypes
        hooks.set_axon_ntff_profile_hook(
            _ntff_profile_via_ctypes('/opt/axon/libaxon_pjrt.so'))
        import concourse.bass_utils as bu
        bu.upload_artifacts = lambda tmpdir: "local://" + tmpdir
        _TRACE_READY = True
    except Exception as e:  # tracing is best-effort
        print("trace shim failed:", e)


def _get_nc():
    global _NC
    if _NC is None:
        _NC = build_nc()
    return _NC


def _pack_weights(conv_w, conv_b, w2_prov, b2_prov, w2_alpha, b2_alpha,
                  w2_ad, b2_ad):
    # Block-Toeplitz conv matrix: BIGW[c, yx, o*128 + ij] = conv_w[o, c, di, dj]
    # where (y, x) = (i + di - 1, j + dj - 1) within the 8x16 map.
    bigw = np.zeros((C_IN, PH, PW, 16, PH, PW), dtype=np.float32)
    for di in range(3):
        for dj in range(3):
            i0, i1 = max(0, 1 - di), min(PH, PH + 1 - di)
            j0, j1 = max(0, 1 - dj), min(PW, PW + 1 - dj)
            ii = np.arange(i0, i1)
            jj = np.arange(j0, j1)
            iy = ii + di - 1
            jx = jj + dj - 1
            # index grids broadcasting to (c, o, i, j); y tied to i, x to j
            cg = np.arange(C_IN)[:, None, None, None]
            og = np.arange(16)[None, :, None, None]
            ig = ii[None, None, :, None]
            jg = jj[None, None, None, :]
            yg = iy[None, None, :, None]
            xg = jx[None, None, None, :]
            bigw[cg, yg, xg, og, ig, jg] = conv_w[:, :, di, dj].T[:, :, None, None]
    bigw = bigw.reshape(C_IN, 128, NCONV).astype(ml_dtypes.bfloat16)
    bigwb = np.repeat(conv_b.astype(np.float32), 128).reshape(1, NCONV)
    bigwb = bigwb.astype(ml_dtypes.bfloat16)

    w2p = np.zeros((HID, NSEG, SEGW), dtype=np.float32)
    b2p = np.full((NSEG, SEGW), NEG, dtype=np.float32)
    w2p[:, 0, :PROV] = w2_prov
    b2p[0, :PROV] = b2_prov
    w2p[:, 1, :ALPHA] = w2_alpha
    b2p[1, :ALPHA] = b2_alpha
    for k in range(6):
        w2p[:, 2 + k, :AD] = w2_ad[k]
        b2p[2 + k, :AD] = b2_ad[k]
    return bigw, bigwb, w2p, b2p


def kernel(x, rois, detection_targets, classification_targets,
           conv_w, conv_b, w1, b1, w2_prov, b2_prov,
           w2_alpha, b2_alpha, w2_ad, b2_ad):
    x = np.ascontiguousarray(np.asarray(x, dtype=np.float32))
    rois = np.ascontiguousarray(np.asarray(rois, dtype=np.int32))
    det = np.asarray(detection_targets, dtype=np.int32)
    cls = np.asarray(classification_targets, dtype=np.int32)
    bigw, bigwb, w2p, b2p = _pack_weights(
        np.asarray(conv_w, np.float32), np.asarray(conv_b, np.float32),
        np.asarray(w2_prov, np.float32), np.asarray(b2_prov, np.float32),
        np.asarray(w2_alpha, np.float32), np.asarray(b2_alpha, np.float32),
        np.asarray(w2_ad, np.float32), np.asarray(b2_ad, np.float32))
    w1f = np.ascontiguousarray(np.asarray(w1, np.float32))
    b1f = np.ascontiguousarray(np.asarray(b1, np.float32))

    in_maps = []
    for c in range(N_CORES):
        sl = slice(c * R, (c + 1) * R)
        det_rep = np.repeat(det[sl], BANDS).astype(np.float32).reshape(M, 1)
        cls_rep = np.repeat(cls[sl], BANDS, axis=0).astype(np.float32)
        in_maps.append({
            "x": x, "rois": rois[sl],
            "det_t": np.ascontiguousarray(det_rep),
            "cls_t": np.ascontiguousarray(cls_rep),
            "bigw": bigw, "bigwb": bigwb,
            "w1": w1f, "b1": b1f, "w2p": w2p, "b2p": b2p,
        })

    nc = _get_nc()
    trace = os.environ.get("KERNEL_TRACE", "0") == "1"
    if trace:
        _install_trace_shim()
    res = run_bass_kernel_spmd(nc, in_maps, core_ids=list(range(N_CORES)),
                               trace=trace)
    global LAST_EXEC_NS
    LAST_EXEC_NS = res.exec_time_ns

    out_prov = np.concatenate([r["o_prov"] for r in res.results], axis=0)
    out_alpha = np.concatenate([r["o_alpha"] for r in res.results], axis=0)
    out_ad = np.concatenate([r["o_ad"] for r in res.results], axis=1)
    mask = np.concatenate([r["o_mask"][:, 0] for r in res.results], axis=0) > 0.5
    total = np.float32(sum(float(r["o_total"][0, 0]) for r in res.results))
    return out_prov, out_alpha, out_ad, mask, total


# revision 17
# speedup vs baseline: 1.1831x; 1.1831x over previous
"""Bass/Trainium2 kernel for nn_CombinedModel (ROI extract + pool + conv +
multi-head classifier + CE loss), data-parallel over ROIs on 8 NeuronCores.

Self-contained: hardcodes shapes; takes full unsharded inputs, returns the
full output tuple (out_prov, out_alpha, out_ad, mask, total).
"""
import sys
for _p in ('/opt/pypackages', '/opt/trn_rl_repo'):
    if _p not in sys.path:
        sys.path.insert(0, _p)

import os
import types

import numpy as np
import ml_dtypes

import concourse.bacc as bacc
import concourse.tile as tile
import concourse.mybir as mybir
import concourse.bass as bass
from concourse.bass import ds
from concourse.bass_utils import run_bass_kernel_spmd
from concourse.masks import make_identity

F32 = mybir.dt.float32
BF16 = mybir.dt.bfloat16
I32 = mybir.dt.int32
AF = mybir.ActivationFunctionType
ALU = mybir.AluOpType
AX = mybir.AxisListType

# Problem constants
N_CORES = 8
N_ROIS = 1024
R = N_ROIS // N_CORES          # 128 rois per core
ROI_H, ROI_W = 112, 224
BANDS = 7
M = R * BANDS                  # 896 band-rows per core
C_IN = 3
PH, PW = 8, 16                 # adaptive pool output (per band)
KIN = C_IN * PH * PW           # 384  (conv matmul contraction, + 1 bias row)
NCONV = 16 * PH * PW           # 2048 (conv output: 16 oc x 128 spatial)
FLAT = 512                     # 16 oc x 4 x 8 after 2x2 pool
HID = 128
NSEG = 8                       # prov, alpha, ad0..5
SEGW = 38                      # padded per-segment logit width
PROV, ALPHA, AD = 38, 25, 35
CONF = 0.25
NEG = -1.0e30

ROIS_PER_TILE = 18             # 18*7 = 126 partitions per crop tile
CROP_TILES = [ROIS_PER_TILE] * 7 + [R - 7 * ROIS_PER_TILE]   # 7x18 + 2
MT = M // 128                  # 7 m-tiles of 128

IMG_HW = 1024 * 1024
MAX_OFF = (1024 - ROI_H - 1) * 1024 + (1024 - ROI_W - 1)


def build_nc():
    nc = bacc.Bacc("TRN2", target_bir_lowering=False, debug=False)

    x = nc.dram_tensor("x", [1, C_IN, 1024, 1024], F32, kind="ExternalInput")
    rois = nc.dram_tensor("rois", [R, 4], I32, kind="ExternalInput")
    det_t = nc.dram_tensor("det_t", [M, 1], F32, kind="ExternalInput")
    cls_t = nc.dram_tensor("cls_t", [M, NSEG], F32, kind="ExternalInput")
    bigw = nc.dram_tensor("bigw", [C_IN, 128, NCONV], BF16, kind="ExternalInput")
    bigwb = nc.dram_tensor("bigwb", [1, NCONV], BF16, kind="ExternalInput")
    w1 = nc.dram_tensor("w1", [8, FLAT, HID], F32, kind="ExternalInput")
    b1 = nc.dram_tensor("b1", [8, HID], F32, kind="ExternalInput")
    w2p = nc.dram_tensor("w2p", [HID, NSEG, SEGW], F32, kind="ExternalInput")
    b2p = nc.dram_tensor("b2p", [NSEG, SEGW], F32, kind="ExternalInput")

    o_prov = nc.dram_tensor("o_prov", [M, PROV], F32, kind="ExternalOutput")
    o_alpha = nc.dram_tensor("o_alpha", [M, ALPHA], F32, kind="ExternalOutput")
    o_ad = nc.dram_tensor("o_ad", [6, M, AD], F32, kind="ExternalOutput")
    o_mask = nc.dram_tensor("o_mask", [M, 1], F32, kind="ExternalOutput")
    o_total = nc.dram_tensor("o_total", [1, 1], F32, kind="ExternalOutput")

    from contextlib import ExitStack
    with tile.TileContext(nc) as tc, ExitStack() as ctx:
        consts = ctx.enter_context(tc.tile_pool(name="consts", bufs=1))
        crop_p = ctx.enter_context(tc.tile_pool(name="crop", bufs=2))
        wred_p = ctx.enter_context(tc.tile_pool(name="wred", bufs=2))
        pool_p = ctx.enter_context(tc.tile_pool(name="pooled", bufs=2))
        big_p = ctx.enter_context(tc.tile_pool(name="big", bufs=1))
        work_p = ctx.enter_context(tc.tile_pool(name="work", bufs=3))
        ce_p = ctx.enter_context(tc.tile_pool(name="ce", bufs=3))
        ps_conv = ctx.enter_context(tc.tile_pool(name="psc", bufs=2, space="PSUM"))
        ps_tr = ctx.enter_context(tc.tile_pool(name="pst", bufs=2, space="PSUM"))
        ps_h = ctx.enter_context(tc.tile_pool(name="psh", bufs=2, space="PSUM"))
        ps_lg = ctx.enter_context(tc.tile_pool(name="psl", bufs=2, space="PSUM"))

        # ---------- constants / weights into SBUF ----------
        ident = consts.tile([128, 128], F32)
        make_identity(nc, ident)

        rois_sb = consts.tile([1, R * 4], I32)
        nc.gpsimd.dma_start(out=rois_sb[:], in_=rois[:].rearrange("r f -> (r f)")[None, :])
        offs = consts.tile([1, R], I32)
        rv = rois_sb[:].rearrange("one (r f) -> one r f", f=4)
        tmp_y = consts.tile([1, R], I32)
        nc.vector.tensor_scalar(
            out=tmp_y[:][:, :, None], in0=rv[:, :, 1:2],
            scalar1=10, scalar2=None, op0=ALU.arith_shift_left)
        nc.vector.tensor_tensor(
            out=offs[:][:, :, None],
            in0=tmp_y[:][:, :, None],
            in1=rv[:, :, 0:1], op=ALU.add)

        bigw_sb = consts.tile([128, C_IN, NCONV], BF16)
        nc.gpsimd.dma_start(out=bigw_sb[:], in_=bigw[:].rearrange("c d n -> d c n"))
        bigwb_sb = consts.tile([1, NCONV], BF16)
        nc.gpsimd.dma_start(out=bigwb_sb[:], in_=bigwb[:])
        w1_sb = consts.tile([128, 8, 4, HID], F32)
        nc.gpsimd.dma_start(out=w1_sb[:], in_=w1[:].rearrange("k (q d) h -> d k q h", d=128))
        b1_sb = consts.tile([1, 8, HID], F32)
        nc.gpsimd.dma_start(out=b1_sb[:], in_=b1[:].rearrange("k h -> (k h)")[None, :].rearrange("one (k h) -> one k h", k=8))
        w2_sb = consts.tile([128, NSEG, SEGW], F32)
        nc.gpsimd.dma_start(out=w2_sb[:], in_=w2p[:])
        b2_sb = consts.tile([128, NSEG, SEGW], F32)
        nc.gpsimd.dma_start(
            out=b2_sb[:],
            in_=bass.AP(tensor=b2p, offset=0,
                        ap=[[0, 128], [SEGW, NSEG], [1, SEGW]]))

        ones_bf = consts.tile([1, M], BF16)
        nc.vector.memset(ones_bf[:], 1.0)
        ones_f = consts.tile([1, M], F32)
        nc.vector.memset(ones_f[:], 1.0)
        ones_col = consts.tile([128, 1], F32)
        nc.vector.memset(ones_col[:], 1.0)

        iota_i = consts.tile([128, SEGW], I32)
        nc.gpsimd.iota(iota_i[:], pattern=[[1, SEGW]], base=0, channel_multiplier=0)
        iota_f = consts.tile([128, SEGW], F32)
        nc.vector.tensor_copy(iota_f[:], iota_i[:])

        acc = consts.tile([128, 1], F32)
        nc.vector.memset(acc[:], 0.0)

        pooledT = big_p.tile([128, C_IN, M], BF16)
        flatT = big_p.tile([128, 4, M], F32)

        # ---------- stage A/B/C: gather crops, pool, transpose ----------
        xf = x[:].rearrange("one c h w -> (one c) (h w)")
        m_base = 0
        for t, nroi in enumerate(CROP_TILES):
            pc = nroi * BANDS
            # crop layout per band-row partition: [r16, c, w] so the whole
            # per-partition free range is contiguous -> 1 DMA per ROI.
            crop = crop_p.tile([126, 16, C_IN, ROI_W], F32, tag="crop")
            js = list(range(nroi))
            for eng, mine in ((nc.sync, js[0::2]), (nc.scalar, js[1::2])):
                for ch in range(0, len(mine), 5):
                    group = mine[ch:ch + 5]
                    regs = []
                    for j in group:
                        r_idx = t * ROIS_PER_TILE + j
                        reg = eng.alloc_register(f"off_{r_idx}")
                        eng.reg_load(reg, offs[0:1, r_idx:r_idx + 1])
                        regs.append(reg)
                    for j, reg in zip(group, regs):
                        off = eng.snap(reg, donate=True, min_val=0,
                                       max_val=MAX_OFF)
                        src = xf[:, ds(off, ROI_H * 1024)]
                        # [(b r) merged, c, w<=224] : 3 dims, balances with
                        # the contiguous dst
                        src = src.rearrange("c (br w) -> br c w", w=1024)
                        src = src[:, :, 0:ROI_W]
                        src = src.rearrange("(b r) c w -> b r c w", b=BANDS)
                        eng.dma_start(out=crop[j * BANDS:(j + 1) * BANDS],
                                      in_=src)

            # pool: 14-wide max then row-pair max -> [pc, 8, 3, 16]
            wred = wred_p.tile([126, 16, C_IN, PW], F32, tag="wred")
            nc.vector.reduce_max(
                out=wred[:pc].rearrange("p r c w -> p (r c) w"), axis=AX.X,
                in_=crop[:pc].rearrange("p r c (w k) -> p (r c) w k", k=14))
            pooled = pool_p.tile([126, C_IN, PH, PW], F32, tag="pooled")
            wv = wred[:pc].rearrange("p (i rp) c w -> p i rp c w", rp=2)
            nc.vector.tensor_tensor(
                out=pooled[:pc].rearrange("p c i w -> p i c w"),
                in0=wv[:, :, 0, :, :], in1=wv[:, :, 1, :, :],
                op=ALU.max)

            # transpose each channel chunk: [pc, 128] -> [128, pc]
            for q in range(C_IN):
                trp = ps_tr.tile([128, 128], F32, tag="tr")
                nc.tensor.transpose(
                    out=trp[:, :pc],
                    in_=pooled[:pc, q].rearrange("p i w -> p (i w)"),
                    identity=ident[:pc, :pc])
                nc.scalar.copy(out=pooledT[:, q, m_base:m_base + pc],
                               in_=trp[:, :pc])
            m_base += pc

        # ---------- stage D: conv as K=385 matmul + 2x2 pool + transpose ----------
        for mt in range(MT):
            ms = mt * 128
            for n in range(4):   # N chunks of 512 over NCONV=2048
                cps = ps_conv.tile([128, 512], F32, tag="cps")
                for q in range(C_IN):
                    nc.tensor.matmul(
                        out=cps[:], lhsT=pooledT[:, q, ms:ms + 128],
                        rhs=bigw_sb[:, q, n * 512:(n + 1) * 512],
                        start=(q == 0), stop=False)
                nc.tensor.matmul(
                    out=cps[:], lhsT=ones_bf[0:1, ms:ms + 128],
                    rhs=bigwb_sb[0:1, n * 512:(n + 1) * 512],
                    start=False, stop=True)
                # 2x2 maxpool over (i,j): [128m, 4o,8i,16j] -> [128m, 4o,4i,8j]
                t1 = work_p.tile([128, 4, PH, PW // 2], F32, tag="t1")
                cv = cps[:].rearrange("p (o i w rp) -> p o i w rp", o=4, i=PH, w=PW // 2)
                nc.vector.tensor_copy(t1[:], cv[:, :, :, :, 0])
                nc.vector.tensor_tensor(
                    out=t1[:], in0=cv[:, :, :, :, 1], in1=t1[:],
                    op=ALU.max)
                t2 = work_p.tile([128, 4, PH // 2, PW // 2], F32, tag="t2")
                tv = t1[:].rearrange("p o (i rp) w -> p o i rp w", rp=2)
                nc.vector.tensor_tensor(
                    out=t2[:], in0=tv[:, :, :, 0, :], in1=tv[:, :, :, 1, :],
                    op=ALU.max)
                # transpose [128m, 128d] -> [128d, 128m], relu on copy-out
                trp2 = ps_tr.tile([128, 128], F32, tag="tr")
                nc.tensor.transpose(
                    out=trp2[:], in_=t2[:].rearrange("p o i w -> p (o i w)"),
                    identity=ident[:])
                nc.scalar.activation(out=flatT[:, n, ms:ms + 128], in_=trp2[:],
                                     func=AF.Relu)

        # ---------- stage E: h = relu(flat @ w1[k] + b1[k]) for 8 heads ----------
        for mt in range(MT):
            ms = mt * 128
            hT = work_p.tile([128, 8, 128], F32, tag="hT")
            for g in range(2):           # head groups of 4 per PSUM bank
                hps = ps_h.tile([128, 512], F32, tag="hps")
                for kk in range(4):
                    k = g * 4 + kk
                    for q in range(4):
                        nc.tensor.matmul(
                            out=hps[:, kk * 128:(kk + 1) * 128],
                            lhsT=w1_sb[:, k, q, :], rhs=flatT[:, q, ms:ms + 128],
                            start=(q == 0), stop=False)
                    nc.tensor.matmul(
                        out=hps[:, kk * 128:(kk + 1) * 128],
                        lhsT=b1_sb[0:1, k, :], rhs=ones_f[0:1, ms:ms + 128],
                        start=False, stop=True)
                nc.scalar.activation(
                    out=hT[:].rearrange("p k h -> p (k h)")[:, g * 512:(g + 1) * 512],
                    in_=hps[:], func=AF.Relu)

            # ---------- stage F: head logits (padded to [8, 38]) ----------
            lps = ps_lg.tile([128, NSEG, SEGW], F32, tag="lps")
            for s in range(NSEG):
                nc.tensor.matmul(
                    out=lps[:, s, :], lhsT=hT[:, s, :], rhs=w2_sb[:, s, :],
                    start=True, stop=True)
            logits = ce_p.tile([128, NSEG, SEGW], F32, tag="logits")
            nc.vector.tensor_tensor(out=logits[:], in0=lps[:], in1=b2_sb[:],
                                    op=ALU.add)

            # outputs
            nc.gpsimd.dma_start(out=o_prov[ms:ms + 128, :], in_=logits[:, 0, 0:PROV])
            nc.gpsimd.dma_start(out=o_alpha[ms:ms + 128, :], in_=logits[:, 1, 0:ALPHA])
            for k in range(6):
                nc.gpsimd.dma_start(
                    out=o_ad[k, ms:ms + 128, :],
                    in_=logits[:, 2 + k, 0:AD])

            # ---------- stage G: CE / mask / loss ----------
            dt_sb = ce_p.tile([128, 1], F32, tag="dt")
            nc.gpsimd.dma_start(out=dt_sb[:], in_=det_t[ms:ms + 128, :])
            ct_sb = ce_p.tile([128, NSEG], F32, tag="ct")
            nc.gpsimd.dma_start(out=ct_sb[:], in_=cls_t[ms:ms + 128, :])

            mx = ce_p.tile([128, NSEG], F32, tag="mx")
            nc.vector.reduce_max(out=mx[:], in_=logits[:], axis=AX.X)
            cent = ce_p.tile([128, NSEG, SEGW], F32, tag="cent")
            nc.vector.tensor_tensor(
                out=cent[:], in0=logits[:],
                in1=mx[:][:, :, None].to_broadcast([128, NSEG, SEGW]),
                op=ALU.subtract)
            exps = ce_p.tile([128, NSEG, SEGW], F32, tag="exps")
            nc.scalar.activation(out=exps[:], in_=cent[:], func=AF.Exp)
            se = ce_p.tile([128, NSEG], F32, tag="se")
            nc.vector.reduce_sum(out=se[:], in_=exps[:], axis=AX.X)
            lnse = ce_p.tile([128, NSEG], F32, tag="lnse")
            nc.scalar.activation(out=lnse[:], in_=se[:], func=AF.Ln)
            lse = ce_p.tile([128, NSEG], F32, tag="lse")
            nc.vector.tensor_tensor(out=lse[:], in0=lnse[:], in1=mx[:], op=ALU.add)

            iob = iota_f[:][:, None, :].to_broadcast([128, NSEG, SEGW])
            oh = ce_p.tile([128, NSEG, SEGW], F32, tag="oh")
            nc.vector.tensor_tensor(
                out=oh[:], in0=iob,
                in1=ct_sb[:][:, :, None].to_broadcast([128, NSEG, SEGW]),
                op=ALU.is_equal)
            ohl = ce_p.tile([128, NSEG, SEGW], F32, tag="ohl")
            nc.vector.tensor_tensor(out=ohl[:], in0=oh[:], in1=logits[:], op=ALU.mult)
            picked = ce_p.tile([128, NSEG], F32, tag="picked")
            nc.vector.reduce_sum(out=picked[:], in_=ohl[:], axis=AX.X)

            # det loss on prov segment
            ohd = ce_p.tile([128, SEGW], F32, tag="ohd")
            nc.vector.tensor_tensor(
                out=ohd[:], in0=iota_f[:],
                in1=dt_sb[:].to_broadcast([128, SEGW]), op=ALU.is_equal)
            ohdl = ce_p.tile([128, SEGW], F32, tag="ohdl")
            nc.vector.tensor_tensor(out=ohdl[:], in0=ohd[:], in1=logits[:, 0, :],
                                    op=ALU.mult)
            pd = ce_p.tile([128, 1], F32, tag="pd")
            nc.vector.reduce_sum(out=pd[:], in_=ohdl[:], axis=AX.X)
            det_loss = ce_p.tile([128, 1], F32, tag="det_loss")
            nc.vector.tensor_tensor(out=det_loss[:], in0=lse[:, 0:1], in1=pd[:],
                                    op=ALU.subtract)
            msk = ce_p.tile([128, 1], F32, tag="msk")
            nc.vector.tensor_scalar(out=msk[:], in0=det_loss[:], scalar1=CONF,
                                    scalar2=None, op0=ALU.is_lt)
            nc.gpsimd.dma_start(out=o_mask[ms:ms + 128, :], in_=msk[:])

            celt = ce_p.tile([128, NSEG], F32, tag="celt")
            nc.vector.tensor_tensor(out=celt[:], in0=lse[:], in1=picked[:],
                                    op=ALU.subtract)
            cls_loss = ce_p.tile([128, 1], F32, tag="cls_loss")
            nc.vector.reduce_sum(out=cls_loss[:], in_=celt[:], axis=AX.X)
            contrib = ce_p.tile([128, 1], F32, tag="contrib")
            nc.vector.tensor_tensor(out=contrib[:], in0=msk[:], in1=cls_loss[:],
                                    op=ALU.mult)
            nc.vector.tensor_tensor(out=acc[:], in0=acc[:], in1=contrib[:],
                                    op=ALU.add)

        # ---------- stage H: partition-sum of acc -> total ----------
        tps = ps_tr.tile([128, 1], F32, tag="tr")
        nc.tensor.matmul(out=tps[0:1, :], lhsT=acc[:], rhs=ones_col[:],
                         start=True, stop=True)
        tot_sb = ce_p.tile([1, 1], F32, tag="tot_sb")
        nc.vector.tensor_copy(tot_sb[:], tps[0:1, :])
        nc.sync.dma_start(out=o_total[:], in_=tot_sb[:])

    nc.compile()
    return nc


_NC = None
LAST_EXEC_NS = None
_TRACE_READY = False


def _install_trace_shim():
    """Enable NTFF profiling under axon (no antenv.axon_hooks in this image)."""
    global _TRACE_READY
    if _TRACE_READY:
        return
    try:
        hooks = types.ModuleType("antenv.axon_hooks")
        hooks._h = None
        hooks.set_axon_ntff_profile_hook = lambda h: setattr(hooks, "_h", h)
        hooks.get_axon_ntff_profile_hook = lambda: hooks._h
        sys.modules["antenv.axon_hooks"] = hooks
        import antenv
        antenv.axon_hooks = hooks
        from trn_agent_boot.trn_boot import _ntff_profile_via_ctypes
        hooks.set_axon_ntff_profile_hook(
            _ntff_profile_via_ctypes('/opt/axon/libaxon_pjrt.so'))
        import concourse.bass_utils as bu
        bu.upload_artifacts = lambda tmpdir: "local://" + tmpdir
        _TRACE_READY = True
    except Exception as e:  # tracing is best-effort
        print("trace shim failed:", e)


def _get_nc():
    global _NC
    if _NC is None:
        _NC = build_nc()
    return _NC


def _pack_weights(conv_w, conv_b, w2_prov, b2_prov, w2_alpha, b2_alpha,
                  w2_ad, b2_ad):
    # Block-Toeplitz conv matrix: BIGW[c, yx, o*128 + ij] = conv_w[o, c, di, dj]
    # where (y, x) = (i + di - 1, j + dj - 1) within the 8x16 map.
    bigw = np.zeros((C_IN, PH, PW, 16, PH, PW), dtype=np.float32)
    for di in range(3):
        for dj in range(3):
            i0, i1 = max(0, 1 - di), min(PH, PH + 1 - di)
            j0, j1 = max(0, 1 - dj), min(PW, PW + 1 - dj)
            ii = np.arange(i0, i1)
            jj = np.arange(j0, j1)
            iy = ii + di - 1
            jx = jj + dj - 1
            # index grids broadcasting to (c, o, i, j); y tied to i, x to j
            cg = np.arange(C_IN)[:, None, None, None]
            og = np.arange(16)[None, :, None, None]
            ig = ii[None, None, :, None]
            jg = jj[None, None, None, :]
            yg = iy[None, None, :, None]
            xg = jx[None, None, None, :]
            bigw[cg, yg, xg, og, ig, jg] = conv_w[:, :, di, dj].T[:, :, None, None]
    bigw = bigw.reshape(C_IN, 128, NCONV).astype(ml_dtypes.bfloat16)
    bigwb = np.repeat(conv_b.astype(np.float32), 128).reshape(1, NCONV)
    bigwb = bigwb.astype(ml_dtypes.bfloat16)

    w2p = np.zeros((HID, NSEG, SEGW), dtype=np.float32)
    b2p = np.full((NSEG, SEGW), NEG, dtype=np.float32)
    w2p[:, 0, :PROV] = w2_prov
    b2p[0, :PROV] = b2_prov
    w2p[:, 1, :ALPHA] = w2_alpha
    b2p[1, :ALPHA] = b2_alpha
    for k in range(6):
        w2p[:, 2 + k, :AD] = w2_ad[k]
        b2p[2 + k, :AD] = b2_ad[k]
    return bigw, bigwb, w2p, b2p


def kernel(x, rois, detection_targets, classification_targets,
           conv_w, conv_b, w1, b1, w2_prov, b2_prov,
           w2_alpha, b2_alpha, w2_ad, b2_ad):
    x = np.ascontiguousarray(np.asarray(x, dtype=np.float32))
    rois = np.ascontiguousarray(np.asarray(rois, dtype=np.int32))
    det = np.asarray(detection_targets, dtype=np.int32)
    cls = np.asarray(classification_targets, dtype=np.int32)
    bigw, bigwb, w2p, b2p = _pack_weights(
        np.asarray(conv_w, np.float32), np.asarray(conv_b, np.float32),
        np.asarray(w2_prov, np.float32), np.asarray(b2_prov, np.float32),
        np.asarray(w2_alpha, np.float32), np.asarray(b2_alpha, np.float32),
        np.asarray(w2_ad, np.float32), np.asarray(b2_ad, np.float32))
    w1f = np.ascontiguousarray(np.asarray(w1, np.float32))
    b1f = np.ascontiguousarray(np.asarray(b1, np.float32))

    in_maps = []
    for c in range(N_CORES):
        sl = slice(c * R, (c + 1) * R)
        det_rep = np.repeat(det[sl], BANDS).astype(np.float32).reshape(M, 1)
        cls_rep = np.repeat(cls[sl], BANDS, axis=0).astype(np.float32)
        in_maps.append({
            "x": x, "rois": rois[sl],
            "det_t": np.ascontiguousarray(det_rep),
            "cls_t": np.ascontiguousarray(cls_rep),
            "bigw": bigw, "bigwb": bigwb,
            "w1": w1f, "b1": b1f, "w2p": w2p, "b2p": b2p,
        })

    nc = _get_nc()
    trace = os.environ.get("KERNEL_TRACE", "0") == "1"
    if trace:
        _install_trace_shim()
    res = run_bass_kernel_spmd(nc, in_maps, core_ids=list(range(N_CORES)),
                               trace=trace)
    global LAST_EXEC_NS
    LAST_EXEC_NS = res.exec_time_ns

    out_prov = np.concatenate([r["o_prov"] for r in res.results], axis=0)
    out_alpha = np.concatenate([r["o_alpha"] for r in res.results], axis=0)
    out_ad = np.concatenate([r["o_ad"] for r in res.results], axis=1)
    mask = np.concatenate([r["o_mask"][:, 0] for r in res.results], axis=0) > 0.5
    total = np.float32(sum(float(r["o_total"][0, 0]) for r in res.results))
    return out_prov, out_alpha, out_ad, mask, total
